# revision 6
# baseline (speedup 1.0000x reference)
"""Multi-head attention (B=4, S=2048, D=1024, H=16, dk=dv=64) on 8 TRN2 cores.

Sharding: core c = 2*b + hg handles batch b = c//2 and heads
[hg*8, hg*8+8). Each core computes a partial output
(its 8 heads' contribution through Wo); the host adds the two partials
per batch.

Per-core device pipeline (matmul inputs bf16, PSUM accumulation fp32,
softmax sums/reciprocal fp32):
  1. qhT/khT projections: lhsT = Wq pair-block [D-chunk 128, 128],
     rhs = qT chunk [128, 512 tok] -> qhT[pair] = [128 (2 heads x 64 dk), 2048].
  2. vh projection: lhsT = vT chunk [128, 128 tok], rhs = Wv [128, 512]
     -> vh[tok, 8*64]; stored per key-chunk as [128, 8*65] bf16 with a
     mask/ones column appended per head (masked keys zeroed).
  3. scores^T per head pair via 64x128 PE tiling: tile (0,0) computes
     head h0 (dk rows 0-63), tile (64,0) head h1. Output [128 keys, 512 q]
     fp32 in PSUM, two key-chunks per PSUM tile ([128, 1024]) so each
     ScalarE exp instruction covers 2 banks.
  4. exp on ScalarE PSUM->SBUF bf16.
  5. mix^T + softmax sums in one matmul: lhsT = vh_aug [64 keys, 65]
     (col 64 = mask), rhs = exp chunk half [64, 512]; tiles (0,0)/(64,0)
     accumulate keys 0-63 / 64-127 into two PSUM banks over 16 chunks.
  6. normalize: madd = bankA + bankB (DVE, fp32); reciprocal of row 64
     (DVE); broadcast the recip row across partitions with an SBUF->SBUF
     DMA; multiply (DVE, out bf16). h1's normalized tile is DMA-shifted
     to partitions 64-127 so each pair's mix^T is one [128, 512] tile
     (e on partitions).
  7. out += mixT_norm.T @ Wo: dense K=128 bf16 matmuls accumulating over
     the 4 pairs; DVE evac fp32 -> DMA to HBM.
"""

import numpy as np

B, S, D = 4, 2048, 1024
H, DK, DV = 16, 64, 64
HC = 8          # heads per core
NP = HC // 2    # head pairs per core
NCORES = 8
NC_CHUNKS = D // 128    # 8 contraction chunks over D
NKC = S // 128          # 16 key chunks
NQB = S // 512          # 4 query blocks
VW = HC * 128           # vh storage: 128 cols per head (dv | mask | zeros)

_COMPILED = {}


def _build_nc():
    import concourse.tile as tile
    from concourse import bacc, mybir
    from contextlib import ExitStack

    F32 = mybir.dt.float32
    BF16 = mybir.dt.bfloat16
    EXP = mybir.ActivationFunctionType.Exp

    nc = bacc.Bacc("TRN2", target_bir_lowering=False, debug=False,
                   num_devices=NCORES)

    qT = nc.dram_tensor("qT", [D, S], BF16, kind="ExternalInput").ap()
    kT = nc.dram_tensor("kT", [D, S], BF16, kind="ExternalInput").ap()
    vT = nc.dram_tensor("vT", [D, S], BF16, kind="ExternalInput").ap()
    wq = nc.dram_tensor("wq", [D, HC * DK], BF16, kind="ExternalInput").ap()
    wk = nc.dram_tensor("wk", [D, HC * DK], BF16, kind="ExternalInput").ap()
    wv = nc.dram_tensor("wv", [D, HC * DV], BF16, kind="ExternalInput").ap()
    wo = nc.dram_tensor("wo", [HC * DV, D], BF16, kind="ExternalInput").ap()
    maskr = nc.dram_tensor("maskr", [128, NKC], F32, kind="ExternalInput").ap()
    out = nc.dram_tensor("out", [S, D], F32, kind="ExternalOutput").ap()

    with tile.TileContext(nc) as tc:
        with ExitStack() as ctx:
            const_pool = ctx.enter_context(tc.tile_pool(name="const", bufs=1))
            w_pool = ctx.enter_context(tc.tile_pool(name="weights", bufs=1))
            act_pool = ctx.enter_context(tc.tile_pool(name="acts", bufs=1))

            mask_sb = const_pool.tile([128, NKC], F32)
            nc.sync.dma_start(mask_sb[:], maskr[:])
            ones_sb = const_pool.tile([128, 64], BF16)
            nc.vector.memset(ones_sb[:], 1.0)

            wv_sb = w_pool.tile([128, NC_CHUNKS * 512], BF16, tag="wv")
            wo_sb = w_pool.tile([128, NP * 1024], BF16, tag="wo")
            for c in range(NC_CHUNKS):
                nc.sync.dma_start(wv_sb[:, c * 512:(c + 1) * 512],
                                  wv[c * 128:(c + 1) * 128, :])
            for p in range(NP):
                nc.sync.dma_start(wo_sb[:, p * 1024:(p + 1) * 1024],
                                  wo[p * 128:(p + 1) * 128, :])

            # persistent activations
            qhT = [act_pool.tile([128, S], BF16, tag=f"qhT{p}", name=f"qhT{p}")
                   for p in range(NP)]
            khT = [act_pool.tile([128, S], BF16, tag=f"khT{p}",
                                 name=f"khT{p}") for p in range(NP)]
            # K=1 bcast helper at partition 64: ones row
            e65_sb = const_pool.tile([128, DV + 1], BF16)
            nc.vector.memset(e65_sb[64:65, :], 1.0)
            vhs = [act_pool.tile([128, VW], BF16, tag=f"vh{t}", name=f"vh{t}")
                   for t in range(NKC)]

            # ---- projections of q and k ----
            with ExitStack() as pctx:
                pj_pool = pctx.enter_context(
                    tc.tile_pool(name="pjpsum", bufs=4, space="PSUM"))
                qk_ctx = ExitStack()
                wqk_pool = qk_ctx.enter_context(tc.tile_pool(name="wqk", bufs=1))
                st_pool = qk_ctx.enter_context(
                    tc.tile_pool(name="stage", bufs=2 * NC_CHUNKS))
                wq_sb = wqk_pool.tile([128, NC_CHUNKS * 512], BF16, tag="wq")
                wk_sb = wqk_pool.tile([128, NC_CHUNKS * 512], BF16, tag="wk")
                for c in range(NC_CHUNKS):
                    nc.sync.dma_start(wq_sb[:, c * 512:(c + 1) * 512],
                                      wq[c * 128:(c + 1) * 128, :])
                    nc.sync.dma_start(wk_sb[:, c * 512:(c + 1) * 512],
                                      wk[c * 128:(c + 1) * 128, :])

                for which, src, wsb in (("q", qT, wq_sb), ("k", kT, wk_sb)):
                    for qb in range(NQB):
                        stg = []
                        for c in range(NC_CHUNKS):
                            t = st_pool.tile([128, 512], BF16, tag="stage",
                                             name=f"stg{c}")
                            nc.sync.dma_start(
                                t[:], src[c * 128:(c + 1) * 128,
                                          qb * 512:(qb + 1) * 512])
                            stg.append(t)
                        for p in range(NP):
                            ps = pj_pool.tile([128, 512], F32, tag="pj")
                            for c in range(NC_CHUNKS):
                                nc.tensor.matmul(
                                    ps[:],
                                    lhsT=wsb[:, c * 512 + p * 128:
                                             c * 512 + (p + 1) * 128],
                                    rhs=stg[c][:],
                                    start=(c == 0), stop=(c == NC_CHUNKS - 1))
                            qsl = slice(qb * 512, (qb + 1) * 512)
                            if which == "q":
                                nc.vector.tensor_copy(qhT[p][:, qsl], ps[:])
                            else:
                                nc.vector.tensor_copy(khT[p][:, qsl], ps[:])

                # ---- projection of v (with mask fold + ones col) ----
                qk_ctx.close()   # free q/k staging + Wq/Wk sbuf
                vt_pool = pctx.enter_context(tc.tile_pool(name="vtpool", bufs=1))
                vt_sb = []
                for c in range(NC_CHUNKS):
                    t = vt_pool.tile([128, S], BF16, tag=f"vt{c}", name=f"vt{c}")
                    for tb in range(NQB):
                        nc.sync.dma_start(
                            t[:, tb * 512:(tb + 1) * 512],
                            vT[c * 128:(c + 1) * 128, tb * 512:(tb + 1) * 512])
                    vt_sb.append(t)
                for t in range(NKC):
                    ps = pj_pool.tile([128, 512], F32, tag="pj")
                    for c in range(NC_CHUNKS):
                        nc.tensor.matmul(
                            ps[:],
                            lhsT=vt_sb[c][:, t * 128:(t + 1) * 128],
                            rhs=wv_sb[:, c * 512:(c + 1) * 512],
                            start=(c == 0), stop=(c == NC_CHUNKS - 1))
                    # masked copy into vh store (strided per head) + mask col
                    dst_dv = vhs[t][:, 0:VW].rearrange(
                        "p (h x) -> p h x", x=128)[:, :, 0:DV]
                    src_dv = ps[:].rearrange("p (h x) -> p h x", x=DV)
                    nc.vector.tensor_scalar_mul(dst_dv, src_dv,
                                                mask_sb[:, t:t + 1])
                    dst_m = vhs[t][:, 0:VW].rearrange(
                        "p (h x) -> p h x", x=128)[:, :, DV:DV + 1]
                    src_m = ones_sb[:, 0:HC].rearrange("p (h x) -> p h x", x=1)
                    nc.vector.tensor_scalar_mul(dst_m, src_m,
                                                mask_sb[:, t:t + 1])

            # ---- attention + output projection ----
            sc_pool = ctx.enter_context(
                tc.tile_pool(name="scpsum", bufs=2, space="PSUM"))
            mx_pool = ctx.enter_context(
                tc.tile_pool(name="mxpsum", bufs=4, space="PSUM"))
            exp_pool = ctx.enter_context(tc.tile_pool(name="exp", bufs=4))
            norm_pool = ctx.enter_context(tc.tile_pool(name="norm", bufs=2 * NP))
            tmp_pool = ctx.enter_context(tc.tile_pool(name="tmp", bufs=4))
            out_pool = ctx.enter_context(tc.tile_pool(name="outsb", bufs=4))

            for qb in range(NQB):
                normT = []
                for p in range(NP):
                    h0, h1 = 2 * p, 2 * p + 1
                    mixP = mx_pool.tile([128, 512], F32, tag="mx")
                    mixR = mx_pool.tile([128, 512], F32, tag="mx")
                    qsl = slice(qb * 512, (qb + 1) * 512)
                    for g in range(NKC // 2):
                        sc0 = sc_pool.tile([128, 1024], F32, tag="sc")
                        sc1 = sc_pool.tile([128, 1024], F32, tag="sc")
                        for s2 in range(2):
                            kc = 2 * g + s2
                            ksl = slice(kc * 128, (kc + 1) * 128)
                            # 64x128 PE row tiling: both heads concurrent
                            nc.tensor.matmul(
                                sc0[:, s2 * 512:(s2 + 1) * 512],
                                lhsT=khT[p][0:64, ksl],
                                rhs=qhT[p][0:64, qsl],
                                start=True, stop=True,
                                tile_position=(0, 0))
                            nc.tensor.matmul(
                                sc1[:, s2 * 512:(s2 + 1) * 512],
                                lhsT=khT[p][64:128, ksl],
                                rhs=qhT[p][64:128, qsl],
                                start=True, stop=True,
                                tile_position=(64, 0))
                        ex0 = exp_pool.tile([128, 1024], BF16, tag="exp")
                        ex1 = exp_pool.tile([128, 1024], BF16, tag="exp")
                        nc.scalar.activation(ex0[:], sc0[:], EXP)
                        nc.scalar.activation(ex1[:], sc1[:], EXP)
                        for s2 in range(2):
                            kc = 2 * g + s2
                            va = vhs[kc]
                            l0 = slice(h0 * 128, h0 * 128 + DV + 1)
                            l1 = slice(h1 * 128, h1 * 128 + DV + 1)
                            esl = slice(s2 * 512, (s2 + 1) * 512)
                            st = (kc == 0)
                            sp = (kc == NKC - 1)
                            nc.tensor.matmul(
                                mixP[0:DV + 1, :],
                                lhsT=va[:, l0], rhs=ex0[:, esl],
                                start=st, stop=sp)
                            nc.tensor.matmul(
                                mixR[0:DV + 1, :],
                                lhsT=va[:, l1], rhs=ex1[:, esl],
                                start=st, stop=sp)
                    # normalize: Z row (part. 64) -> bf16, K=1 PE bcast,
                    # reciprocal at base partition 0 (custom-DVE ops
                    # misbehave at base partition 64), scale mix rows
                    nt = norm_pool.tile([128, 512], BF16, tag="norm")
                    normT.append(nt)
                    zrow = tmp_pool.tile([128, 1024], BF16, tag="zrow")
                    nc.vector.tensor_copy(zrow[64:65, 0:512], mixP[64:65, :])
                    nc.vector.tensor_copy(zrow[64:65, 512:1024],
                                          mixR[64:65, :])
                    bc0 = mx_pool.tile([128, 512], F32, tag="mx")
                    bc1 = mx_pool.tile([128, 512], F32, tag="mx")
                    nc.tensor.matmul(
                        bc0[0:64, :], lhsT=e65_sb[64:65, 0:64],
                        rhs=zrow[64:65, 0:512], start=True, stop=True,
                        tile_position=(64, 0))
                    nc.tensor.matmul(
                        bc1[0:64, :], lhsT=e65_sb[64:65, 0:64],
                        rhs=zrow[64:65, 512:1024], start=True, stop=True,
                        tile_position=(64, 0))
                    rec0 = tmp_pool.tile([64, 512], F32, tag="rec")
                    rec1 = tmp_pool.tile([64, 512], F32, tag="rec")
                    nc.vector.reciprocal_approx_fast(rec0[:], bc0[0:64, :])
                    nc.vector.reciprocal_approx_fast(rec1[:], bc1[0:64, :])
                    nc.vector.tensor_mul(nt[0:64, :], mixP[0:64, :],
                                         rec0[:])
                    sh1 = tmp_pool.tile([64, 512], BF16, tag="sh1")
                    nc.vector.tensor_mul(sh1[:], mixR[0:64, :],
                                         rec1[:])
                    nc.sync.dma_start(nt[64:128, :], sh1[:])

                # ---- Wo ----
                for tt in range(4):
                    for dh in range(2):
                        wps = mx_pool.tile([128, 512], F32, tag="mx")
                        for p in range(NP):
                            nc.tensor.matmul(
                                wps[:],
                                lhsT=normT[p][:, tt * 128:(tt + 1) * 128],
                                rhs=wo_sb[:, p * 1024 + dh * 512:
                                          p * 1024 + (dh + 1) * 512],
                                start=(p == 0), stop=(p == NP - 1))
                        osb = out_pool.tile([128, 512], F32, tag="osb")
                        nc.vector.tensor_copy(osb[:], wps[:])
                        nc.sync.dma_start(
                            out[qb * 512 + tt * 128:qb * 512 + (tt + 1) * 128,
                                dh * 512:(dh + 1) * 512], osb[:])

    nc.compile()
    return nc


def _get_nc():
    if "nc" not in _COMPILED:
        _COMPILED["nc"] = _build_nc()
    return _COMPILED["nc"]


def _shard_inputs(q, k, v, mask, Wq, Wk, Wv, Wo):
    """Build the per-core input maps (host-side layout prep)."""
    import ml_dtypes

    bf16 = ml_dtypes.bfloat16
    in_maps = []
    maskf = np.asarray(mask).astype(np.float32)
    q = np.asarray(q, np.float32)
    k = np.asarray(k, np.float32)
    v = np.asarray(v, np.float32)
    Wq = np.asarray(Wq, np.float32)
    Wk = np.asarray(Wk, np.float32)
    Wv = np.asarray(Wv, np.float32)
    Wo = np.asarray(Wo, np.float32)
    scale = np.float32(1.0 / np.sqrt(DK))
    for c in range(NCORES):
        b, hg = c // 2, c % 2
        hs = hg * HC
        m = {
            "qT": np.ascontiguousarray(q[b].T).astype(bf16),
            "kT": np.ascontiguousarray(k[b].T).astype(bf16),
            "vT": np.ascontiguousarray(v[b].T).astype(bf16),
            # head-major col blocks; fold 1/sqrt(dk) into Wq
            "wq": np.ascontiguousarray(
                Wq[hs:hs + HC].transpose(1, 0, 2).reshape(D, HC * DK) * scale
            ).astype(bf16),
            "wk": np.ascontiguousarray(
                Wk[hs:hs + HC].transpose(1, 0, 2).reshape(D, HC * DK)
            ).astype(bf16),
            "wv": np.ascontiguousarray(
                Wv[hs:hs + HC].transpose(1, 0, 2).reshape(D, HC * DV)
            ).astype(bf16),
            "wo": np.ascontiguousarray(Wo[hs * DV:(hs + HC) * DV]).astype(bf16),
            "maskr": np.ascontiguousarray(
                maskf[b].reshape(NKC, 128).T).astype(np.float32),
        }
        in_maps.append(m)
    return in_maps


def kernel(q, k, v, mask, Wq, Wk, Wv, Wo, _trace=False):
    from concourse.bass_utils import run_bass_kernel_spmd

    nc = _get_nc()
    in_maps = _shard_inputs(q, k, v, mask, Wq, Wk, Wv, Wo)
    res = run_bass_kernel_spmd(nc, in_maps, list(range(NCORES)),
                               trace=_trace)
    out = np.zeros((B, S, D), np.float32)
    for c in range(NCORES):
        out[c // 2] += res.results[c]["out"]
    if _trace:
        _COMPILED["last_result"] = res
    return out



# revision 10
# speedup vs baseline: 1.0624x; 1.0624x over previous
"""Multi-head attention (B=4, S=2048, D=1024, H=16, dk=dv=64) on 8 TRN2 cores.

Sharding: core c = 2*b + hg handles batch b = c//2 and heads
[hg*8, hg*8+8). Each core computes a partial output
(its 8 heads' contribution through Wo); the host adds the two partials
per batch.

Per-core device pipeline (matmul inputs bf16, PSUM accumulation fp32,
softmax sums/reciprocal fp32):
  1. qhT/khT projections: lhsT = Wq pair-block [D-chunk 128, 128],
     rhs = qT chunk [128, 512 tok] -> qhT[pair] = [128 (2 heads x 64 dk), 2048].
  2. vh projection: lhsT = vT chunk [128, 128 tok], rhs = Wv [128, 512]
     -> vh[tok, 8*64]; stored per key-chunk as [128, 8*65] bf16 with a
     mask/ones column appended per head (masked keys zeroed).
  3. scores^T per head pair via 64x128 PE tiling: tile (0,0) computes
     head h0 (dk rows 0-63), tile (64,0) head h1. Output [128 keys, 512 q]
     fp32 in PSUM, two key-chunks per PSUM tile ([128, 1024]) so each
     ScalarE exp instruction covers 2 banks.
  4. exp on ScalarE PSUM->SBUF bf16.
  5. mix^T + softmax sums in one matmul: lhsT = vh_aug [64 keys, 65]
     (col 64 = mask), rhs = exp chunk half [64, 512]; tiles (0,0)/(64,0)
     accumulate keys 0-63 / 64-127 into two PSUM banks over 16 chunks.
  6. normalize: madd = bankA + bankB (DVE, fp32); reciprocal of row 64
     (DVE); broadcast the recip row across partitions with an SBUF->SBUF
     DMA; multiply (DVE, out bf16). h1's normalized tile is DMA-shifted
     to partitions 64-127 so each pair's mix^T is one [128, 512] tile
     (e on partitions).
  7. out += mixT_norm.T @ Wo: dense K=128 bf16 matmuls accumulating over
     the 4 pairs; DVE evac fp32 -> DMA to HBM.
"""

import numpy as np

B, S, D = 4, 2048, 1024
H, DK, DV = 16, 64, 64
HC = 8          # heads per core
NP = HC // 2    # head pairs per core
NCORES = 8
NC_CHUNKS = D // 128    # 8 contraction chunks over D
NKC = S // 128          # 16 key chunks
NQB = S // 512          # 4 query blocks
VW = HC * 128           # vh storage: 128 cols per head (dv | mask | zeros)

_COMPILED = {}


def _build_nc():
    import concourse.tile as tile
    from concourse import bacc, mybir
    from contextlib import ExitStack

    F32 = mybir.dt.float32
    BF16 = mybir.dt.bfloat16
    EXP = mybir.ActivationFunctionType.Exp

    nc = bacc.Bacc("TRN2", target_bir_lowering=False, debug=False,
                   num_devices=NCORES)

    qT = nc.dram_tensor("qT", [D, S], BF16, kind="ExternalInput").ap()
    kT = nc.dram_tensor("kT", [D, S], BF16, kind="ExternalInput").ap()
    vT = nc.dram_tensor("vT", [D, S], BF16, kind="ExternalInput").ap()
    wq = nc.dram_tensor("wq", [D, HC * DK], BF16, kind="ExternalInput").ap()
    wk = nc.dram_tensor("wk", [D, HC * DK], BF16, kind="ExternalInput").ap()
    wv = nc.dram_tensor("wv", [D, HC * DV], BF16, kind="ExternalInput").ap()
    wo = nc.dram_tensor("wo", [HC * DV, D], BF16, kind="ExternalInput").ap()
    maskr = nc.dram_tensor("maskr", [128, NKC], F32, kind="ExternalInput").ap()
    out = nc.dram_tensor("out", [S, D], F32, kind="ExternalOutput").ap()

    with tile.TileContext(nc) as tc:
        with ExitStack() as ctx:
            const_pool = ctx.enter_context(tc.tile_pool(name="const", bufs=1))
            w_pool = ctx.enter_context(tc.tile_pool(name="weights", bufs=1))
            act_pool = ctx.enter_context(tc.tile_pool(name="acts", bufs=1))

            mask_sb = const_pool.tile([128, NKC], F32)
            nc.sync.dma_start(mask_sb[:], maskr[:])
            ones_sb = const_pool.tile([128, 64], BF16)
            nc.vector.memset(ones_sb[:], 1.0)

            wv_sb = w_pool.tile([128, NC_CHUNKS * 512], BF16, tag="wv")
            wo_sb = w_pool.tile([128, NP * 1024], BF16, tag="wo")

            # persistent activations
            qhT = [act_pool.tile([128, S], BF16, tag=f"qhT{p}", name=f"qhT{p}")
                   for p in range(NP)]
            khT0 = [act_pool.tile([128, S], BF16, tag=f"khT0{p}",
                                  name=f"khT0{p}") for p in range(NP)]
            khT1 = [act_pool.tile([128, S], BF16, tag=f"khT1{p}",
                                  name=f"khT1{p}") for p in range(NP)]
            for p in range(NP):
                nc.vector.memset(khT0[p][64:128, :], 0.0)
                nc.vector.memset(khT1[p][0:64, :], 0.0)
            # K=1 bcast helper at partition 64: ones row
            e65_sb = const_pool.tile([128, DV + 1], BF16)
            nc.vector.memset(e65_sb[64:65, :], 1.0)
            vhs = [act_pool.tile([128, VW], BF16, tag=f"vh{t}", name=f"vh{t}")
                   for t in range(NKC)]
            for t in range(NKC):
                nc.vector.memset(vhs[t][:, :], 0.0)

            # ---- projections of q and k ----
            with ExitStack() as pctx:
                pj_pool = pctx.enter_context(
                    tc.tile_pool(name="pjpsum", bufs=4, space="PSUM"))
                qk_ctx = ExitStack()
                wqk_pool = qk_ctx.enter_context(tc.tile_pool(name="wqk", bufs=1))
                st_pool = qk_ctx.enter_context(
                    tc.tile_pool(name="stage", bufs=2 * NC_CHUNKS))
                wq_sb = wqk_pool.tile([128, NC_CHUNKS * 512], BF16, tag="wq")
                wk_sb = wqk_pool.tile([128, NC_CHUNKS * 512], BF16, tag="wk")
                for c in range(NC_CHUNKS):
                    nc.sync.dma_start(wq_sb[:, c * 512:(c + 1) * 512],
                                      wq[c * 128:(c + 1) * 128, :])
                    nc.sync.dma_start(wk_sb[:, c * 512:(c + 1) * 512],
                                      wk[c * 128:(c + 1) * 128, :])

                for which, src, wsb in (("q", qT, wq_sb), ("k", kT, wk_sb)):
                    for qb in range(NQB):
                        stg = []
                        for c in range(NC_CHUNKS):
                            t = st_pool.tile([128, 512], BF16, tag="stage",
                                             name=f"stg{c}")
                            nc.sync.dma_start(
                                t[:], src[c * 128:(c + 1) * 128,
                                          qb * 512:(qb + 1) * 512])
                            stg.append(t)
                        for p in range(NP):
                            ps = pj_pool.tile([128, 512], F32, tag="pj")
                            for c in range(NC_CHUNKS):
                                nc.tensor.matmul(
                                    ps[:],
                                    lhsT=wsb[:, c * 512 + p * 128:
                                             c * 512 + (p + 1) * 128],
                                    rhs=stg[c][:],
                                    start=(c == 0), stop=(c == NC_CHUNKS - 1))
                            qsl = slice(qb * 512, (qb + 1) * 512)
                            if which == "q":
                                nc.vector.tensor_copy(qhT[p][:, qsl], ps[:])
                            else:
                                nc.vector.tensor_copy(khT0[p][0:64, qsl],
                                                      ps[0:64, :])
                                nc.vector.tensor_copy(khT1[p][64:128, qsl],
                                                      ps[64:128, :])
                    if which == "q":
                        # prefetch v/o weights behind the q staging traffic
                        for c in range(NC_CHUNKS):
                            nc.sync.dma_start(wv_sb[:, c * 512:(c + 1) * 512],
                                              wv[c * 128:(c + 1) * 128, :])
                        for p in range(NP):
                            nc.sync.dma_start(
                                wo_sb[:, p * 1024:(p + 1) * 1024],
                                wo[p * 128:(p + 1) * 128, :])

                # ---- projection of v (with mask fold + ones col) ----
                qk_ctx.close()   # free q/k staging + Wq/Wk sbuf
                vt_pool = pctx.enter_context(tc.tile_pool(name="vtpool", bufs=1))
                vt_sb = []
                for c in range(NC_CHUNKS):
                    t = vt_pool.tile([128, S], BF16, tag=f"vt{c}", name=f"vt{c}")
                    for tb in range(NQB):
                        nc.sync.dma_start(
                            t[:, tb * 512:(tb + 1) * 512],
                            vT[c * 128:(c + 1) * 128, tb * 512:(tb + 1) * 512])
                    vt_sb.append(t)
                for t in range(NKC):
                    ps = pj_pool.tile([128, 512], F32, tag="pj")
                    for c in range(NC_CHUNKS):
                        nc.tensor.matmul(
                            ps[:],
                            lhsT=vt_sb[c][:, t * 128:(t + 1) * 128],
                            rhs=wv_sb[:, c * 512:(c + 1) * 512],
                            start=(c == 0), stop=(c == NC_CHUNKS - 1))
                    # masked copy into vh store (strided per head) + mask col
                    dst_dv = vhs[t][:, 0:VW].rearrange(
                        "p (h x) -> p h x", x=128)[:, :, 0:DV]
                    src_dv = ps[:].rearrange("p (h x) -> p h x", x=DV)
                    nc.vector.tensor_scalar_mul(dst_dv, src_dv,
                                                mask_sb[:, t:t + 1])
                    dst_m = vhs[t][:, 0:VW].rearrange(
                        "p (h x) -> p h x", x=128)[:, :, DV:DV + 1]
                    src_m = ones_sb[:, 0:HC].rearrange("p (h x) -> p h x", x=1)
                    nc.vector.tensor_scalar_mul(dst_m, src_m,
                                                mask_sb[:, t:t + 1])

            # ---- attention + output projection ----
            sc_pool = ctx.enter_context(
                tc.tile_pool(name="scpsum", bufs=2, space="PSUM"))
            mx_pool = ctx.enter_context(
                tc.tile_pool(name="mxpsum", bufs=4, space="PSUM"))
            exp_pool = ctx.enter_context(tc.tile_pool(name="exp", bufs=4))
            norm_pool = ctx.enter_context(tc.tile_pool(name="norm", bufs=2 * NP))
            tmp_pool = ctx.enter_context(tc.tile_pool(name="tmp", bufs=4))
            out_pool = ctx.enter_context(tc.tile_pool(name="outsb", bufs=4))

            for qb in range(NQB):
                normT = []
                for p in range(NP):
                    h0, h1 = 2 * p, 2 * p + 1
                    mixP = mx_pool.tile([128, 512], F32, tag="mx")
                    mixR = mx_pool.tile([128, 512], F32, tag="mx")
                    qsl = slice(qb * 512, (qb + 1) * 512)
                    for g in range(NKC // 2):
                        sc0 = sc_pool.tile([128, 1024], F32, tag="sc")
                        sc1 = sc_pool.tile([128, 1024], F32, tag="sc")
                        for s2 in range(2):
                            kc = 2 * g + s2
                            ksl = slice(kc * 128, (kc + 1) * 128)
                            nc.tensor.matmul(
                                sc0[:, s2 * 512:(s2 + 1) * 512],
                                lhsT=khT0[p][:, ksl], rhs=qhT[p][:, qsl],
                                start=True, stop=True)
                            nc.tensor.matmul(
                                sc1[:, s2 * 512:(s2 + 1) * 512],
                                lhsT=khT1[p][:, ksl], rhs=qhT[p][:, qsl],
                                start=True, stop=True)
                        ex0 = exp_pool.tile([128, 1024], BF16, tag="exp")
                        ex1 = exp_pool.tile([128, 1024], BF16, tag="exp")
                        nc.scalar.activation(ex0[:], sc0[:], EXP)
                        nc.scalar.activation(ex1[:], sc1[:], EXP)
                        for s2 in range(2):
                            kc = 2 * g + s2
                            va = vhs[kc]
                            l0 = slice(h0 * 128, (h0 + 1) * 128)
                            l1 = slice(h1 * 128, (h1 + 1) * 128)
                            esl = slice(s2 * 512, (s2 + 1) * 512)
                            st = (kc == 0)
                            sp = (kc == NKC - 1)
                            nc.tensor.matmul(
                                mixP[:, :],
                                lhsT=va[:, l0], rhs=ex0[:, esl],
                                start=st, stop=sp)
                            nc.tensor.matmul(
                                mixR[:, :],
                                lhsT=va[:, l1], rhs=ex1[:, esl],
                                start=st, stop=sp)
                    # normalize: Z row (part. 64) -> bf16, K=1 PE bcast,
                    # reciprocal at base partition 0 (custom-DVE ops
                    # misbehave at base partition 64), scale mix rows
                    nt = norm_pool.tile([128, 512], BF16, tag="norm")
                    normT.append(nt)
                    zrow = tmp_pool.tile([128, 1024], BF16, tag="zrow")
                    nc.vector.tensor_copy(zrow[64:65, 0:512], mixP[64:65, :])
                    nc.vector.tensor_copy(zrow[64:65, 512:1024],
                                          mixR[64:65, :])
                    bc0 = mx_pool.tile([128, 512], F32, tag="mx")
                    bc1 = mx_pool.tile([128, 512], F32, tag="mx")
                    nc.tensor.matmul(
                        bc0[0:64, :], lhsT=e65_sb[64:65, 0:64],
                        rhs=zrow[64:65, 0:512], start=True, stop=True,
                        tile_position=(64, 0))
                    nc.tensor.matmul(
                        bc1[0:64, :], lhsT=e65_sb[64:65, 0:64],
                        rhs=zrow[64:65, 512:1024], start=True, stop=True,
                        tile_position=(64, 0))
                    rec0 = tmp_pool.tile([64, 512], F32, tag="rec")
                    rec1 = tmp_pool.tile([64, 512], F32, tag="rec")
                    nc.vector.reciprocal_approx_fast(rec0[:], bc0[0:64, :])
                    nc.vector.reciprocal_approx_fast(rec1[:], bc1[0:64, :])
                    nc.vector.tensor_mul(nt[0:64, :], mixP[0:64, :],
                                         rec0[:])
                    sh1 = tmp_pool.tile([64, 512], BF16, tag="sh1")
                    nc.vector.tensor_mul(sh1[:], mixR[0:64, :],
                                         rec1[:])
                    nc.sync.dma_start(nt[64:128, :], sh1[:])

                # ---- Wo ----
                for tt in range(4):
                    for dh in range(2):
                        wps = mx_pool.tile([128, 512], F32, tag="mx")
                        for p in range(NP):
                            nc.tensor.matmul(
                                wps[:],
                                lhsT=normT[p][:, tt * 128:(tt + 1) * 128],
                                rhs=wo_sb[:, p * 1024 + dh * 512:
                                          p * 1024 + (dh + 1) * 512],
                                start=(p == 0), stop=(p == NP - 1))
                        osb = out_pool.tile([128, 512], F32, tag="osb")
                        nc.vector.tensor_copy(osb[:], wps[:])
                        nc.sync.dma_start(
                            out[qb * 512 + tt * 128:qb * 512 + (tt + 1) * 128,
                                dh * 512:(dh + 1) * 512], osb[:])

    nc.compile()
    return nc


def _get_nc():
    if "nc" not in _COMPILED:
        _COMPILED["nc"] = _build_nc()
    return _COMPILED["nc"]


def _shard_inputs(q, k, v, mask, Wq, Wk, Wv, Wo):
    """Build the per-core input maps (host-side layout prep)."""
    import ml_dtypes

    bf16 = ml_dtypes.bfloat16
    in_maps = []
    maskf = np.asarray(mask).astype(np.float32)
    q = np.asarray(q, np.float32)
    k = np.asarray(k, np.float32)
    v = np.asarray(v, np.float32)
    Wq = np.asarray(Wq, np.float32)
    Wk = np.asarray(Wk, np.float32)
    Wv = np.asarray(Wv, np.float32)
    Wo = np.asarray(Wo, np.float32)
    scale = np.float32(1.0 / np.sqrt(DK))
    for c in range(NCORES):
        b, hg = c // 2, c % 2
        hs = hg * HC
        m = {
            "qT": np.ascontiguousarray(q[b].T).astype(bf16),
            "kT": np.ascontiguousarray(k[b].T).astype(bf16),
            "vT": np.ascontiguousarray(v[b].T).astype(bf16),
            # head-major col blocks; fold 1/sqrt(dk) into Wq
            "wq": np.ascontiguousarray(
                Wq[hs:hs + HC].transpose(1, 0, 2).reshape(D, HC * DK) * scale
            ).astype(bf16),
            "wk": np.ascontiguousarray(
                Wk[hs:hs + HC].transpose(1, 0, 2).reshape(D, HC * DK)
            ).astype(bf16),
            "wv": np.ascontiguousarray(
                Wv[hs:hs + HC].transpose(1, 0, 2).reshape(D, HC * DV)
            ).astype(bf16),
            "wo": np.ascontiguousarray(Wo[hs * DV:(hs + HC) * DV]).astype(bf16),
            "maskr": np.ascontiguousarray(
                maskf[b].reshape(NKC, 128).T).astype(np.float32),
        }
        in_maps.append(m)
    return in_maps


def kernel(q, k, v, mask, Wq, Wk, Wv, Wo, _trace=False):
    from concourse.bass_utils import run_bass_kernel_spmd

    nc = _get_nc()
    in_maps = _shard_inputs(q, k, v, mask, Wq, Wk, Wv, Wo)
    res = run_bass_kernel_spmd(nc, in_maps, list(range(NCORES)),
                               trace=_trace)
    out = np.zeros((B, S, D), np.float32)
    for c in range(NCORES):
        out[c // 2] += res.results[c]["out"]
    if _trace:
        _COMPILED["last_result"] = res
    return out



# revision 11
# speedup vs baseline: 1.0859x; 1.0221x over previous
"""Multi-head attention (B=4, S=2048, D=1024, H=16, dk=dv=64) on 8 TRN2 cores.

Sharding: core c = 2*b + hg handles batch b = c//2 and heads
[hg*8, hg*8+8). Each core computes a partial output
(its 8 heads' contribution through Wo); the host adds the two partials
per batch.

Per-core device pipeline (matmul inputs bf16, PSUM accumulation fp32,
softmax sums/reciprocal fp32):
  1. q(qb=0) projection first (shortest path to attention), then khT
     projections (pair layout: h0 dk on partitions 0-63, h1 on 64-127),
     then vh projection per key-chunk as [128, 8*128] bf16 with a
     mask/ones column appended per head (masked keys zeroed; cols 65-127
     zero). q(qb+1) projections are interleaved into attention qb.
  2. scores^T per head pair via 64x128 PE row tiling: per key-chunk one
     [128, 1024] PSUM tile holds h0 scores (cols 0-511, tile (0,0)) and
     h1 scores (cols 512-1023, tile (64,0)); the two matmuls co-stream
     in the PE array (separate PSUM banks).
  3. exp on ScalarE PSUM->SBUF bf16, one [128, 1024] ACTIVATE per chunk.
  4. mix^T + softmax sums in one matmul: lhsT = vh block [128 keys,
     128] (col 64 = mask/ones), rhs = exp half [128, 512]; PSUM
     accumulation over the 16 chunks (mixP for h0, mixR for h1).
  5. normalize: Z row (partition 64) -> bf16 SBUF, K=1 PE matmul
     broadcasts it to partitions 0-63, reciprocal_approx_fast at base
     partition 0 (custom-DVE ops misbehave at base partition 64),
     multiply mix rows by 1/Z (bf16 out). h1's normalized tile is
     DMA-shifted to partitions 64-127 so each pair's mix^T is one
     [128, 512] tile (e on partitions).
  6. out += mixT_norm.T @ Wo: dense K=128 bf16 matmuls accumulating over
     the 4 pairs; DVE evac fp32 -> DMA to HBM.
"""

import numpy as np

B, S, D = 4, 2048, 1024
H, DK, DV = 16, 64, 64
HC = 8          # heads per core
NP = HC // 2    # head pairs per core
NCORES = 8
NC_CHUNKS = D // 128    # 8 contraction chunks over D
NKC = S // 128          # 16 key chunks
NQB = S // 512          # 4 query blocks
VW = HC * 128           # vh storage: 128 cols per head (dv | mask | zeros)

_COMPILED = {}


def _build_nc():
    import concourse.tile as tile
    from concourse import bacc, mybir
    from contextlib import ExitStack

    F32 = mybir.dt.float32
    BF16 = mybir.dt.bfloat16
    EXP = mybir.ActivationFunctionType.Exp

    nc = bacc.Bacc("TRN2", target_bir_lowering=False, debug=False,
                   num_devices=NCORES)

    qT = nc.dram_tensor("qT", [D, S], BF16, kind="ExternalInput").ap()
    kT = nc.dram_tensor("kT", [D, S], BF16, kind="ExternalInput").ap()
    vT = nc.dram_tensor("vT", [D, S], BF16, kind="ExternalInput").ap()
    wq = nc.dram_tensor("wq", [D, HC * DK], BF16, kind="ExternalInput").ap()
    wk = nc.dram_tensor("wk", [D, HC * DK], BF16, kind="ExternalInput").ap()
    wv = nc.dram_tensor("wv", [D, HC * DV], BF16, kind="ExternalInput").ap()
    wo = nc.dram_tensor("wo", [HC * DV, D], BF16, kind="ExternalInput").ap()
    maskr = nc.dram_tensor("maskr", [128, NKC], F32, kind="ExternalInput").ap()
    out = nc.dram_tensor("out", [S, D], F32, kind="ExternalOutput").ap()

    with tile.TileContext(nc) as tc:
        with ExitStack() as ctx:
            const_pool = ctx.enter_context(tc.tile_pool(name="const", bufs=1))
            w_pool = ctx.enter_context(tc.tile_pool(name="weights", bufs=1))
            act_pool = ctx.enter_context(tc.tile_pool(name="acts", bufs=1))
            st_pool = ctx.enter_context(
                tc.tile_pool(name="stage", bufs=2 * NC_CHUNKS))
            vt_pool = ctx.enter_context(tc.tile_pool(name="vtpool", bufs=1))
            # PSUM: pj(2, shared with bc) + sc(2x2) + mx(2) = 8 banks
            pj_pool = ctx.enter_context(
                tc.tile_pool(name="pjpsum", bufs=2, space="PSUM"))
            sc_pool = ctx.enter_context(
                tc.tile_pool(name="scpsum", bufs=2, space="PSUM"))
            mx_pool = ctx.enter_context(
                tc.tile_pool(name="mxpsum", bufs=2, space="PSUM"))
            exp_pool = ctx.enter_context(tc.tile_pool(name="exp", bufs=4))
            norm_pool = ctx.enter_context(tc.tile_pool(name="norm",
                                                       bufs=2 * NP))
            tmp_pool = ctx.enter_context(tc.tile_pool(name="tmp", bufs=4))
            out_pool = ctx.enter_context(tc.tile_pool(name="outsb", bufs=4))

            # weight tiles (DMAs issued in need-order below)
            wq_sb = w_pool.tile([128, NC_CHUNKS * 512], BF16, tag="wq")
            wk_sb = w_pool.tile([128, NC_CHUNKS * 512], BF16, tag="wk")
            wv_sb = w_pool.tile([128, NC_CHUNKS * 512], BF16, tag="wv")
            wo_sb = w_pool.tile([128, NP * 1024], BF16, tag="wo")

            mask_sb = const_pool.tile([128, NKC], F32)
            ones_sb = const_pool.tile([128, 64], BF16)
            e65_sb = const_pool.tile([128, DV + 1], BF16)

            # persistent activations
            qhT = [act_pool.tile([128, S], BF16, tag=f"qhT{p}", name=f"qhT{p}")
                   for p in range(NP)]
            khT = [act_pool.tile([128, S], BF16, tag=f"khT{p}",
                                 name=f"khT{p}") for p in range(NP)]
            vhs = [act_pool.tile([128, VW], BF16, tag=f"vh{t}", name=f"vh{t}")
                   for t in range(NKC)]

            # ---- issue order: q(qb0) path first ----
            for c in range(NC_CHUNKS):
                nc.sync.dma_start(wq_sb[:, c * 512:(c + 1) * 512],
                                  wq[c * 128:(c + 1) * 128, :])
            nc.sync.dma_start(mask_sb[:], maskr[:])
            nc.vector.memset(ones_sb[:], 1.0)
            nc.vector.memset(e65_sb[64:65, :], 1.0)
            for t in range(NKC):
                nc.vector.memset(vhs[t][:, :], 0.0)

            def stage_block(src, qb):
                stg = []
                for c in range(NC_CHUNKS):
                    t = st_pool.tile([128, 512], BF16, tag="stage",
                                     name=f"stg{c}")
                    nc.sync.dma_start(
                        t[:], src[c * 128:(c + 1) * 128,
                                  qb * 512:(qb + 1) * 512])
                    stg.append(t)
                return stg

            def proj_pair(stg, wsb, p, dst, qb):
                ps = pj_pool.tile([128, 512], F32, tag="pj")
                for c in range(NC_CHUNKS):
                    nc.tensor.matmul(
                        ps[:],
                        lhsT=wsb[:, c * 512 + p * 128:c * 512 + (p + 1) * 128],
                        rhs=stg[c][:],
                        start=(c == 0), stop=(c == NC_CHUNKS - 1))
                nc.vector.tensor_copy(
                    dst[p][:, qb * 512:(qb + 1) * 512], ps[:])

            # q(qb=0) projection
            stg = stage_block(qT, 0)
            for p in range(NP):
                proj_pair(stg, wq_sb, p, qhT, 0)

            # k projection (all blocks)
            for c in range(NC_CHUNKS):
                nc.sync.dma_start(wk_sb[:, c * 512:(c + 1) * 512],
                                  wk[c * 128:(c + 1) * 128, :])
            for kb in range(NQB):
                stg = stage_block(kT, kb)
                for p in range(NP):
                    proj_pair(stg, wk_sb, p, khT, kb)

            # v projection (with mask fold + ones col)
            for c in range(NC_CHUNKS):
                nc.sync.dma_start(wv_sb[:, c * 512:(c + 1) * 512],
                                  wv[c * 128:(c + 1) * 128, :])
            for p in range(NP):
                nc.sync.dma_start(wo_sb[:, p * 1024:(p + 1) * 1024],
                                  wo[p * 128:(p + 1) * 128, :])
            vt_sb = []
            for c in range(NC_CHUNKS):
                t = vt_pool.tile([128, S], BF16, tag=f"vt{c}", name=f"vt{c}")
                for tb in range(NQB):
                    nc.sync.dma_start(
                        t[:, tb * 512:(tb + 1) * 512],
                        vT[c * 128:(c + 1) * 128, tb * 512:(tb + 1) * 512])
                vt_sb.append(t)
            for t in range(NKC):
                ps = pj_pool.tile([128, 512], F32, tag="pj")
                for c in range(NC_CHUNKS):
                    nc.tensor.matmul(
                        ps[:],
                        lhsT=vt_sb[c][:, t * 128:(t + 1) * 128],
                        rhs=wv_sb[:, c * 512:(c + 1) * 512],
                        start=(c == 0), stop=(c == NC_CHUNKS - 1))
                # masked copy into vh store (strided per head) + mask col
                dst_dv = vhs[t][:, 0:VW].rearrange(
                    "p (h x) -> p h x", x=128)[:, :, 0:DV]
                src_dv = ps[:].rearrange("p (h x) -> p h x", x=DV)
                nc.vector.tensor_scalar_mul(dst_dv, src_dv,
                                            mask_sb[:, t:t + 1])
                dst_m = vhs[t][:, 0:VW].rearrange(
                    "p (h x) -> p h x", x=128)[:, :, DV:DV + 1]
                src_m = ones_sb[:, 0:HC].rearrange("p (h x) -> p h x", x=1)
                nc.vector.tensor_scalar_mul(dst_m, src_m,
                                            mask_sb[:, t:t + 1])

            # ---- attention + output projection ----
            for qb in range(NQB):
                normT = []
                stg_next = stage_block(qT, qb + 1) if qb + 1 < NQB else None
                for p in range(NP):
                    h0, h1 = 2 * p, 2 * p + 1
                    qsl = slice(qb * 512, (qb + 1) * 512)
                    mixP = mx_pool.tile([128, 512], F32, tag="mx")
                    mixR = mx_pool.tile([128, 512], F32, tag="mx")
                    for kc in range(NKC):
                        ksl = slice(kc * 128, (kc + 1) * 128)
                        scP = sc_pool.tile([128, 1024], F32, tag="sc")
                        # 64x128 PE row tiling: both heads co-stream
                        nc.tensor.matmul(
                            scP[:, 0:512],
                            lhsT=khT[p][0:64, ksl],
                            rhs=qhT[p][0:64, qsl],
                            start=True, stop=True,
                            tile_position=(0, 0))
                        nc.tensor.matmul(
                            scP[:, 512:1024],
                            lhsT=khT[p][64:128, ksl],
                            rhs=qhT[p][64:128, qsl],
                            start=True, stop=True,
                            tile_position=(64, 0))
                        exP = exp_pool.tile([128, 1024], BF16, tag="exp")
                        nc.scalar.activation(exP[:], scP[:], EXP)
                        va = vhs[kc]
                        st = (kc == 0)
                        sp = (kc == NKC - 1)
                        nc.tensor.matmul(
                            mixP[:, :],
                            lhsT=va[:, h0 * 128:(h0 + 1) * 128],
                            rhs=exP[:, 0:512],
                            start=st, stop=sp)
                        nc.tensor.matmul(
                            mixR[:, :],
                            lhsT=va[:, h1 * 128:(h1 + 1) * 128],
                            rhs=exP[:, 512:1024],
                            start=st, stop=sp)
                    # normalize: Z rows (part. 64) -> bf16, K=1 PE bcast,
                    # reciprocal at base partition 0 (custom-DVE ops
                    # misbehave at base partition 64), scale mix rows
                    nt = norm_pool.tile([128, 512], BF16, tag="norm")
                    normT.append(nt)
                    zrow = tmp_pool.tile([128, 1024], BF16, tag="zrow")
                    nc.vector.tensor_copy(zrow[64:65, 0:512], mixP[64:65, :])
                    nc.vector.tensor_copy(zrow[64:65, 512:1024],
                                          mixR[64:65, :])
                    bc0 = pj_pool.tile([128, 512], F32, tag="pj")
                    bc1 = pj_pool.tile([128, 512], F32, tag="pj")
                    nc.tensor.matmul(
                        bc0[0:64, :], lhsT=e65_sb[64:65, 0:64],
                        rhs=zrow[64:65, 0:512], start=True, stop=True,
                        tile_position=(64, 0))
                    nc.tensor.matmul(
                        bc1[0:64, :], lhsT=e65_sb[64:65, 0:64],
                        rhs=zrow[64:65, 512:1024], start=True, stop=True,
                        tile_position=(64, 0))
                    rec0 = tmp_pool.tile([64, 512], F32, tag="rec")
                    rec1 = tmp_pool.tile([64, 512], F32, tag="rec")
                    nc.vector.reciprocal_approx_fast(rec0[:], bc0[0:64, :])
                    nc.vector.reciprocal_approx_fast(rec1[:], bc1[0:64, :])
                    nc.vector.tensor_mul(nt[0:64, :], mixP[0:64, :],
                                         rec0[:])
                    sh1 = tmp_pool.tile([64, 512], BF16, tag="sh1")
                    nc.vector.tensor_mul(sh1[:], mixR[0:64, :],
                                         rec1[:])
                    nc.sync.dma_start(nt[64:128, :], sh1[:])
                    # interleave next q-block projection into the bubble
                    if stg_next is not None:
                        proj_pair(stg_next, wq_sb, p, qhT, qb + 1)

                # ---- Wo ----
                for tt in range(4):
                    for dh in range(2):
                        wps = mx_pool.tile([128, 512], F32, tag="mx")
                        for p in range(NP):
                            nc.tensor.matmul(
                                wps[:],
                                lhsT=normT[p][:, tt * 128:(tt + 1) * 128],
                                rhs=wo_sb[:, p * 1024 + dh * 512:
                                          p * 1024 + (dh + 1) * 512],
                                start=(p == 0), stop=(p == NP - 1))
                        osb = out_pool.tile([128, 512], F32, tag="osb")
                        nc.vector.tensor_copy(osb[:], wps[:])
                        nc.sync.dma_start(
                            out[qb * 512 + tt * 128:qb * 512 + (tt + 1) * 128,
                                dh * 512:(dh + 1) * 512], osb[:])

    nc.compile()
    return nc


def _get_nc():
    if "nc" not in _COMPILED:
        _COMPILED["nc"] = _build_nc()
    return _COMPILED["nc"]


def _shard_inputs(q, k, v, mask, Wq, Wk, Wv, Wo):
    """Build the per-core input maps (host-side layout prep)."""
    import ml_dtypes

    bf16 = ml_dtypes.bfloat16
    in_maps = []
    maskf = np.asarray(mask).astype(np.float32)
    q = np.asarray(q, np.float32)
    k = np.asarray(k, np.float32)
    v = np.asarray(v, np.float32)
    Wq = np.asarray(Wq, np.float32)
    Wk = np.asarray(Wk, np.float32)
    Wv = np.asarray(Wv, np.float32)
    Wo = np.asarray(Wo, np.float32)
    scale = np.float32(1.0 / np.sqrt(DK))
    for c in range(NCORES):
        b, hg = c // 2, c % 2
        hs = hg * HC
        m = {
            "qT": np.ascontiguousarray(q[b].T).astype(bf16),
            "kT": np.ascontiguousarray(k[b].T).astype(bf16),
            "vT": np.ascontiguousarray(v[b].T).astype(bf16),
            # head-major col blocks; fold 1/sqrt(dk) into Wq
            "wq": np.ascontiguousarray(
                Wq[hs:hs + HC].transpose(1, 0, 2).reshape(D, HC * DK) * scale
            ).astype(bf16),
            "wk": np.ascontiguousarray(
                Wk[hs:hs + HC].transpose(1, 0, 2).reshape(D, HC * DK)
            ).astype(bf16),
            "wv": np.ascontiguousarray(
                Wv[hs:hs + HC].transpose(1, 0, 2).reshape(D, HC * DV)
            ).astype(bf16),
            "wo": np.ascontiguousarray(Wo[hs * DV:(hs + HC) * DV]).astype(bf16),
            "maskr": np.ascontiguousarray(
                maskf[b].reshape(NKC, 128).T).astype(np.float32),
        }
        in_maps.append(m)
    return in_maps


def kernel(q, k, v, mask, Wq, Wk, Wv, Wo, _trace=False):
    from concourse.bass_utils import run_bass_kernel_spmd

    nc = _get_nc()
    in_maps = _shard_inputs(q, k, v, mask, Wq, Wk, Wv, Wo)
    res = run_bass_kernel_spmd(nc, in_maps, list(range(NCORES)),
                               trace=_trace)
    out = np.zeros((B, S, D), np.float32)
    for c in range(NCORES):
        out[c // 2] += res.results[c]["out"]
    if _trace:
        _COMPILED["last_result"] = res
    return out


# revision 13
# speedup vs baseline: 1.1040x; 1.0166x over previous
"""Multi-head attention (B=4, S=2048, D=1024, H=16, dk=dv=64) on 8 TRN2 cores.

Sharding: core c = 2*b + hg handles batch b = c//2 and heads
[hg*8, hg*8+8). Each core computes a partial output
(its 8 heads' contribution through Wo); the host adds the two partials
per batch.

Per-core device pipeline (matmul inputs bf16, PSUM accumulation fp32,
softmax sums/reciprocal fp32):
  1. q(qb=0) projection first (shortest path to attention), then khT
     projections (pair layout: h0 dk on partitions 0-63, h1 on 64-127),
     then vh projection per key-chunk as [128, 8*128] bf16 with a
     mask/ones column appended per head (masked keys zeroed; cols 65-127
     zero). q(qb+1) projections are interleaved into attention qb.
  2. scores^T per head pair via 64x128 PE row tiling: per key-chunk one
     [128, 1024] PSUM tile holds h0 scores (cols 0-511, tile (0,0)) and
     h1 scores (cols 512-1023, tile (64,0)); the two matmuls co-stream
     in the PE array (separate PSUM banks).
  3. exp on ScalarE PSUM->SBUF bf16, one [128, 1024] ACTIVATE per chunk.
  4. mix^T + softmax sums in one matmul: lhsT = vh block [128 keys,
     128] (col 64 = mask/ones), rhs = exp half [128, 512]; PSUM
     accumulation over the 16 chunks (mixP for h0, mixR for h1).
  5. normalize: Z row (partition 64) -> bf16 SBUF, K=1 PE matmul
     broadcasts it to partitions 0-63, reciprocal_approx_fast at base
     partition 0 (custom-DVE ops misbehave at base partition 64),
     multiply mix rows by 1/Z (bf16 out). h1's normalized tile is
     DMA-shifted to partitions 64-127 so each pair's mix^T is one
     [128, 512] tile (e on partitions).
  6. out += mixT_norm.T @ Wo: dense K=128 bf16 matmuls accumulating over
     the 4 pairs; DVE evac fp32 -> DMA to HBM.
"""

import numpy as np

B, S, D = 4, 2048, 1024
H, DK, DV = 16, 64, 64
HC = 8          # heads per core
NP = HC // 2    # head pairs per core
NCORES = 8
NC_CHUNKS = D // 128    # 8 contraction chunks over D
NKC = S // 128          # 16 key chunks
NQB = S // 512          # 4 query blocks
VW = HC * 128           # vh storage: 128 cols per head (dv | mask | zeros)

_COMPILED = {}


def _build_nc():
    import concourse.tile as tile
    from concourse import bacc, mybir
    from contextlib import ExitStack

    F32 = mybir.dt.float32
    BF16 = mybir.dt.bfloat16
    EXP = mybir.ActivationFunctionType.Exp

    nc = bacc.Bacc("TRN2", target_bir_lowering=False, debug=False,
                   num_devices=NCORES)

    qT = nc.dram_tensor("qT", [D, S], BF16, kind="ExternalInput").ap()
    kT = nc.dram_tensor("kT", [D, S], BF16, kind="ExternalInput").ap()
    vT = nc.dram_tensor("vT", [D, S], BF16, kind="ExternalInput").ap()
    wq = nc.dram_tensor("wq", [D, HC * DK], BF16, kind="ExternalInput").ap()
    wk = nc.dram_tensor("wk", [D, HC * DK], BF16, kind="ExternalInput").ap()
    wv = nc.dram_tensor("wv", [D, HC * DV], BF16, kind="ExternalInput").ap()
    wo = nc.dram_tensor("wo", [HC * DV, D], BF16, kind="ExternalInput").ap()
    maskr = nc.dram_tensor("maskr", [128, NKC], F32, kind="ExternalInput").ap()
    out = nc.dram_tensor("out", [S, D], F32, kind="ExternalOutput").ap()

    with tile.TileContext(nc) as tc:
        with ExitStack() as ctx:
            const_pool = ctx.enter_context(tc.tile_pool(name="const", bufs=1))
            w_pool = ctx.enter_context(tc.tile_pool(name="weights", bufs=1))
            act_pool = ctx.enter_context(tc.tile_pool(name="acts", bufs=1))
            st_pool = ctx.enter_context(
                tc.tile_pool(name="stage", bufs=2 * NC_CHUNKS))
            vt_pool = ctx.enter_context(tc.tile_pool(name="vtpool", bufs=1))
            # PSUM: pj(2, shared with bc) + sc(2x2) + mx(2) = 8 banks
            pj_pool = ctx.enter_context(
                tc.tile_pool(name="pjpsum", bufs=2, space="PSUM"))
            sc_pool = ctx.enter_context(
                tc.tile_pool(name="scpsum", bufs=2, space="PSUM"))
            mx_pool = ctx.enter_context(
                tc.tile_pool(name="mxpsum", bufs=2, space="PSUM"))
            exp_pool = ctx.enter_context(tc.tile_pool(name="exp", bufs=4))
            norm_pool = ctx.enter_context(tc.tile_pool(name="norm",
                                                       bufs=2 * NP))
            tmp_pool = ctx.enter_context(tc.tile_pool(name="tmp", bufs=4))
            out_pool = ctx.enter_context(tc.tile_pool(name="outsb", bufs=4))

            # weight tiles (DMAs issued in need-order below)
            wq_sb = w_pool.tile([128, NC_CHUNKS * 512], BF16, tag="wq")
            wk_sb = w_pool.tile([128, NC_CHUNKS * 512], BF16, tag="wk")
            wv_sb = w_pool.tile([128, NC_CHUNKS * 512], BF16, tag="wv")
            wo_sb = w_pool.tile([128, NP * 1024], BF16, tag="wo")

            mask_sb = const_pool.tile([128, NKC], F32)
            ones_sb = const_pool.tile([128, 64], BF16)
            e65_sb = const_pool.tile([128, DV + 1], BF16)

            # persistent activations
            qhT = [act_pool.tile([128, S], BF16, tag=f"qhT{p}", name=f"qhT{p}")
                   for p in range(NP)]
            khT = [act_pool.tile([128, S], BF16, tag=f"khT{p}",
                                 name=f"khT{p}") for p in range(NP)]
            vhs = [act_pool.tile([128, VW], BF16, tag=f"vh{t}", name=f"vh{t}")
                   for t in range(NKC)]

            # ---- issue order: q(qb0) path first ----
            for c in range(NC_CHUNKS):
                nc.sync.dma_start(wq_sb[:, c * 512:(c + 1) * 512],
                                  wq[c * 128:(c + 1) * 128, :])
            nc.sync.dma_start(mask_sb[:], maskr[:])
            nc.vector.memset(ones_sb[:], 1.0)
            nc.vector.memset(e65_sb[64:65, :], 1.0)
            for t in range(NKC):
                nc.vector.memset(vhs[t][:, :], 0.0)

            def stage_block(src, qb):
                stg = []
                for c in range(NC_CHUNKS):
                    t = st_pool.tile([128, 512], BF16, tag="stage",
                                     name=f"stg{c}")
                    nc.sync.dma_start(
                        t[:], src[c * 128:(c + 1) * 128,
                                  qb * 512:(qb + 1) * 512])
                    stg.append(t)
                return stg

            def proj_pair(stg, wsb, p, dst, qb):
                ps = pj_pool.tile([128, 512], F32, tag="pj")
                for c in range(NC_CHUNKS):
                    nc.tensor.matmul(
                        ps[:],
                        lhsT=wsb[:, c * 512 + p * 128:c * 512 + (p + 1) * 128],
                        rhs=stg[c][:],
                        start=(c == 0), stop=(c == NC_CHUNKS - 1))
                nc.vector.tensor_copy(
                    dst[p][:, qb * 512:(qb + 1) * 512], ps[:])

            # q(qb=0) projection
            stg = stage_block(qT, 0)
            for p in range(NP):
                proj_pair(stg, wq_sb, p, qhT, 0)

            # k projection (all blocks)
            for c in range(NC_CHUNKS):
                nc.sync.dma_start(wk_sb[:, c * 512:(c + 1) * 512],
                                  wk[c * 128:(c + 1) * 128, :])
            for kb in range(NQB):
                stg = stage_block(kT, kb)
                for p in range(NP):
                    proj_pair(stg, wk_sb, p, khT, kb)

            # v projection (with mask fold + ones col)
            for c in range(NC_CHUNKS):
                nc.sync.dma_start(wv_sb[:, c * 512:(c + 1) * 512],
                                  wv[c * 128:(c + 1) * 128, :])
            for p in range(NP):
                nc.sync.dma_start(wo_sb[:, p * 1024:(p + 1) * 1024],
                                  wo[p * 128:(p + 1) * 128, :])
            vt_sb = []
            for c in range(NC_CHUNKS):
                t = vt_pool.tile([128, S], BF16, tag=f"vt{c}", name=f"vt{c}")
                for tb in range(NQB):
                    nc.sync.dma_start(
                        t[:, tb * 512:(tb + 1) * 512],
                        vT[c * 128:(c + 1) * 128, tb * 512:(tb + 1) * 512])
                vt_sb.append(t)
            for t in range(NKC):
                ps = pj_pool.tile([128, 512], F32, tag="pj")
                for c in range(NC_CHUNKS):
                    nc.tensor.matmul(
                        ps[:],
                        lhsT=vt_sb[c][:, t * 128:(t + 1) * 128],
                        rhs=wv_sb[:, c * 512:(c + 1) * 512],
                        start=(c == 0), stop=(c == NC_CHUNKS - 1))
                # masked copy into vh store (strided per head) + mask col
                dst_dv = vhs[t][:, 0:VW].rearrange(
                    "p (h x) -> p h x", x=128)[:, :, 0:DV]
                src_dv = ps[:].rearrange("p (h x) -> p h x", x=DV)
                nc.vector.tensor_scalar_mul(dst_dv, src_dv,
                                            mask_sb[:, t:t + 1])
                dst_m = vhs[t][:, 0:VW].rearrange(
                    "p (h x) -> p h x", x=128)[:, :, DV:DV + 1]
                src_m = ones_sb[:, 0:HC].rearrange("p (h x) -> p h x", x=1)
                nc.vector.tensor_scalar_mul(dst_m, src_m,
                                            mask_sb[:, t:t + 1])

            # ---- attention + output projection ----
            # Software-pipelined over flat units u = (qb, p, kc): the
            # scores+exp issue runs LOOK units ahead of the mix issue so
            # ScalarE keeps exp-ing across pair boundaries while the PE
            # absorbs normalize/proj/Wo work in its slack.
            LOOK = 2
            units = [(qb, p, kc) for qb in range(NQB) for p in range(NP)
                     for kc in range(NKC)]
            pend = {}
            mix_tiles = {}
            stg_nxt = {}
            normT = {qb: [] for qb in range(NQB)}

            def issue_sc(u):
                qb, p, kc = u
                qsl = slice(qb * 512, (qb + 1) * 512)
                ksl = slice(kc * 128, (kc + 1) * 128)
                scP = sc_pool.tile([128, 1024], F32, tag="sc")
                # 64x128 PE row tiling: both heads co-stream
                nc.tensor.matmul(
                    scP[:, 0:512],
                    lhsT=khT[p][0:64, ksl], rhs=qhT[p][0:64, qsl],
                    start=True, stop=True, tile_position=(0, 0))
                nc.tensor.matmul(
                    scP[:, 512:1024],
                    lhsT=khT[p][64:128, ksl], rhs=qhT[p][64:128, qsl],
                    start=True, stop=True, tile_position=(64, 0))
                exP = exp_pool.tile([128, 1024], BF16, tag="exp")
                nc.scalar.activation(exP[:], scP[:], EXP)
                pend[u] = exP

            def issue_mix(u):
                qb, p, kc = u
                h0, h1 = 2 * p, 2 * p + 1
                if kc == 0:
                    mix_tiles[(qb, p)] = (
                        mx_pool.tile([128, 512], F32, tag="mx", name="mixP"),
                        mx_pool.tile([128, 512], F32, tag="mx", name="mixR"))
                mixP, mixR = mix_tiles[(qb, p)]
                exP = pend.pop(u)
                va = vhs[kc]
                st = (kc == 0)
                sp = (kc == NKC - 1)
                nc.tensor.matmul(
                    mixP[:, :], lhsT=va[:, h0 * 128:(h0 + 1) * 128],
                    rhs=exP[:, 0:512], start=st, stop=sp)
                nc.tensor.matmul(
                    mixR[:, :], lhsT=va[:, h1 * 128:(h1 + 1) * 128],
                    rhs=exP[:, 512:1024], start=st, stop=sp)

            def normalize(qb, p):
                # Z rows (part. 64) -> bf16, K=1 PE bcast, reciprocal at
                # base partition 0 (custom-DVE ops misbehave at base
                # partition 64), scale mix rows
                mixP, mixR = mix_tiles.pop((qb, p))
                nt = norm_pool.tile([128, 512], BF16, tag="norm")
                normT[qb].append(nt)
                zrow = tmp_pool.tile([128, 1024], BF16, tag="zrow")
                nc.vector.tensor_copy(zrow[64:65, 0:512], mixP[64:65, :])
                nc.vector.tensor_copy(zrow[64:65, 512:1024], mixR[64:65, :])
                bc0 = pj_pool.tile([128, 512], F32, tag="pj")
                bc1 = pj_pool.tile([128, 512], F32, tag="pj")
                nc.tensor.matmul(
                    bc0[0:64, :], lhsT=e65_sb[64:65, 0:64],
                    rhs=zrow[64:65, 0:512], start=True, stop=True,
                    tile_position=(64, 0))
                nc.tensor.matmul(
                    bc1[0:64, :], lhsT=e65_sb[64:65, 0:64],
                    rhs=zrow[64:65, 512:1024], start=True, stop=True,
                    tile_position=(64, 0))
                rec0 = tmp_pool.tile([64, 512], F32, tag="rec")
                rec1 = tmp_pool.tile([64, 512], F32, tag="rec")
                nc.vector.reciprocal_approx_fast(rec0[:], bc0[0:64, :])
                nc.vector.reciprocal_approx_fast(rec1[:], bc1[0:64, :])
                nc.vector.tensor_mul(nt[0:64, :], mixP[0:64, :], rec0[:])
                sh1 = tmp_pool.tile([64, 512], BF16, tag="sh1")
                nc.vector.tensor_mul(sh1[:], mixR[0:64, :], rec1[:])
                nc.sync.dma_start(nt[64:128, :], sh1[:])

            def wo_block(qb):
                for tt in range(4):
                    for dh in range(2):
                        wps = mx_pool.tile([128, 512], F32, tag="mx")
                        for p in range(NP):
                            nc.tensor.matmul(
                                wps[:],
                                lhsT=normT[qb][p][:, tt * 128:(tt + 1) * 128],
                                rhs=wo_sb[:, p * 1024 + dh * 512:
                                          p * 1024 + (dh + 1) * 512],
                                start=(p == 0), stop=(p == NP - 1))
                        osb = out_pool.tile([128, 512], F32, tag="osb")
                        nc.vector.tensor_copy(osb[:], wps[:])
                        nc.sync.dma_start(
                            out[qb * 512 + tt * 128:qb * 512 + (tt + 1) * 128,
                                dh * 512:(dh + 1) * 512], osb[:])

            for i, u in enumerate(units):
                if i == 0:
                    for j in range(LOOK):
                        issue_sc(units[j])
                issue_mix(u)
                if i + LOOK < len(units):
                    issue_sc(units[i + LOOK])
                qb, p, kc = u
                if p == 0 and kc == 0 and qb + 1 < NQB:
                    stg_nxt[qb + 1] = stage_block(qT, qb + 1)
                if kc == NKC - 1:
                    normalize(qb, p)
                    if qb + 1 < NQB:
                        proj_pair(stg_nxt[qb + 1], wq_sb, p, qhT, qb + 1)
                    if p == NP - 1:
                        wo_block(qb)

    nc.compile()
    return nc


def _get_nc():
    if "nc" not in _COMPILED:
        _COMPILED["nc"] = _build_nc()
    return _COMPILED["nc"]


def _shard_inputs(q, k, v, mask, Wq, Wk, Wv, Wo):
    """Build the per-core input maps (host-side layout prep)."""
    import ml_dtypes

    bf16 = ml_dtypes.bfloat16
    in_maps = []
    maskf = np.asarray(mask).astype(np.float32)
    q = np.asarray(q, np.float32)
    k = np.asarray(k, np.float32)
    v = np.asarray(v, np.float32)
    Wq = np.asarray(Wq, np.float32)
    Wk = np.asarray(Wk, np.float32)
    Wv = np.asarray(Wv, np.float32)
    Wo = np.asarray(Wo, np.float32)
    scale = np.float32(1.0 / np.sqrt(DK))
    for c in range(NCORES):
        b, hg = c // 2, c % 2
        hs = hg * HC
        m = {
            "qT": np.ascontiguousarray(q[b].T).astype(bf16),
            "kT": np.ascontiguousarray(k[b].T).astype(bf16),
            "vT": np.ascontiguousarray(v[b].T).astype(bf16),
            # head-major col blocks; fold 1/sqrt(dk) into Wq
            "wq": np.ascontiguousarray(
                Wq[hs:hs + HC].transpose(1, 0, 2).reshape(D, HC * DK) * scale
            ).astype(bf16),
            "wk": np.ascontiguousarray(
                Wk[hs:hs + HC].transpose(1, 0, 2).reshape(D, HC * DK)
            ).astype(bf16),
            "wv": np.ascontiguousarray(
                Wv[hs:hs + HC].transpose(1, 0, 2).reshape(D, HC * DV)
            ).astype(bf16),
            "wo": np.ascontiguousarray(Wo[hs * DV:(hs + HC) * DV]).astype(bf16),
            "maskr": np.ascontiguousarray(
                maskf[b].reshape(NKC, 128).T).astype(np.float32),
        }
        in_maps.append(m)
    return in_maps


def kernel(q, k, v, mask, Wq, Wk, Wv, Wo, _trace=False):
    from concourse.bass_utils import run_bass_kernel_spmd

    nc = _get_nc()
    in_maps = _shard_inputs(q, k, v, mask, Wq, Wk, Wv, Wo)
    res = run_bass_kernel_spmd(nc, in_maps, list(range(NCORES)),
                               trace=_trace)
    out = np.zeros((B, S, D), np.float32)
    for c in range(NCORES):
        out[c // 2] += res.results[c]["out"]
    if _trace:
        _COMPILED["last_result"] = res
    return out


# revision 16
# speedup vs baseline: 1.1432x; 1.0356x over previous
"""Multi-head attention (B=4, S=2048, D=1024, H=16, dk=dv=64) on 8 TRN2 cores.

Sharding: core c = 2*b + hg handles batch b = c//2 and heads
[hg*8, hg*8+8). Each core computes a partial output
(its 8 heads' contribution through Wo); the host adds the two partials
per batch.

Per-core device pipeline (matmul inputs bf16, PSUM accumulation fp32,
softmax sums/reciprocal fp32):
  1. q(qb=0) projection first (shortest path to attention), then khT
     projections (pair layout: h0 dk on partitions 0-63, h1 on 64-127),
     then vh projection per key-chunk as [128, 8*128] bf16 with a
     mask/ones column appended per head (masked keys zeroed; cols 65-127
     zero). q(qb+1) projections are interleaved into attention qb.
  2. scores^T per head pair via 64x128 PE row tiling: per key-chunk one
     [128, 1024] PSUM tile holds h0 scores (cols 0-511, tile (0,0)) and
     h1 scores (cols 512-1023, tile (64,0)); the two matmuls co-stream
     in the PE array (separate PSUM banks).
  3. exp on ScalarE PSUM->SBUF bf16, one [128, 1024] ACTIVATE per chunk.
  4. mix^T + softmax sums in one matmul: lhsT = vh block [128 keys,
     128] (col 64 = mask/ones), rhs = exp half [128, 512]; PSUM
     accumulation over the 16 chunks (mixP for h0, mixR for h1).
  5. normalize: Z row (partition 64) -> bf16 SBUF, K=1 PE matmul
     broadcasts it to partitions 0-63, reciprocal_approx_fast at base
     partition 0 (custom-DVE ops misbehave at base partition 64),
     multiply mix rows by 1/Z (bf16 out). h1's normalized tile is
     DMA-shifted to partitions 64-127 so each pair's mix^T is one
     [128, 512] tile (e on partitions).
  6. out += mixT_norm.T @ Wo: dense K=128 bf16 matmuls accumulating over
     the 4 pairs; DVE evac fp32 -> DMA to HBM.
"""

import numpy as np

B, S, D = 4, 2048, 1024
H, DK, DV = 16, 64, 64
HC = 8          # heads per core
NP = HC // 2    # head pairs per core
NCORES = 8
NC_CHUNKS = D // 128    # 8 contraction chunks over D
NKC = S // 128          # 16 key chunks
NQB = S // 512          # 4 query blocks
VW = HC * 128           # vh storage: 128 cols per head (dv | mask | zeros)

_COMPILED = {}


def _build_nc():
    import concourse.tile as tile
    from concourse import bacc, mybir
    from contextlib import ExitStack

    F32 = mybir.dt.float32
    BF16 = mybir.dt.bfloat16
    EXP = mybir.ActivationFunctionType.Exp

    nc = bacc.Bacc("TRN2", target_bir_lowering=False, debug=False,
                   num_devices=NCORES)

    qT = nc.dram_tensor("qT", [D, S], BF16, kind="ExternalInput").ap()
    kT = nc.dram_tensor("kT", [D, S], BF16, kind="ExternalInput").ap()
    vT = nc.dram_tensor("vT", [D, S], BF16, kind="ExternalInput").ap()
    wq = nc.dram_tensor("wq", [D, HC * DK], BF16, kind="ExternalInput").ap()
    wk = nc.dram_tensor("wk", [D, HC * DK], BF16, kind="ExternalInput").ap()
    wv = nc.dram_tensor("wv", [D, HC * DV], BF16, kind="ExternalInput").ap()
    wo = nc.dram_tensor("wo", [HC * DV, D], BF16, kind="ExternalInput").ap()
    maskr = nc.dram_tensor("maskr", [128, NKC], F32, kind="ExternalInput").ap()
    out = nc.dram_tensor("out", [S, D], F32, kind="ExternalOutput").ap()

    with tile.TileContext(nc) as tc:
        with ExitStack() as ctx:
            const_pool = ctx.enter_context(tc.tile_pool(name="const", bufs=1))
            w_pool = ctx.enter_context(tc.tile_pool(name="weights", bufs=1))
            act_pool = ctx.enter_context(tc.tile_pool(name="acts", bufs=1))
            st_pool = ctx.enter_context(
                tc.tile_pool(name="stage", bufs=2 * NC_CHUNKS))
            vt_pool = ctx.enter_context(tc.tile_pool(name="vtpool", bufs=1))
            # PSUM: pj(2, shared with bc) + sc(2x2) + mx(2) = 8 banks
            pj_pool = ctx.enter_context(
                tc.tile_pool(name="pjpsum", bufs=2, space="PSUM"))
            sc_pool = ctx.enter_context(
                tc.tile_pool(name="scpsum", bufs=2, space="PSUM"))
            mx_pool = ctx.enter_context(
                tc.tile_pool(name="mxpsum", bufs=1, space="PSUM"))
            exp_pool = ctx.enter_context(tc.tile_pool(name="exp", bufs=4))
            norm_pool = ctx.enter_context(tc.tile_pool(name="norm",
                                                       bufs=2 * NP))
            tmp_pool = ctx.enter_context(tc.tile_pool(name="tmp", bufs=2))
            out_pool = ctx.enter_context(tc.tile_pool(name="outsb", bufs=4))

            # weight tiles (DMAs issued in need-order below)
            wq_sb = w_pool.tile([128, NC_CHUNKS * 512], BF16, tag="wq")
            wk_sb = w_pool.tile([128, NC_CHUNKS * 512], BF16, tag="wk")
            wv_sb = w_pool.tile([128, NC_CHUNKS * 512], BF16, tag="wv")
            wo_sb = w_pool.tile([128, NP * 1024], BF16, tag="wo")

            mask_sb = const_pool.tile([128, NKC], F32)
            ones_sb = const_pool.tile([128, 64], BF16)
            e65_sb = const_pool.tile([128, DV + 1], BF16)

            # persistent activations
            qhT = [act_pool.tile([128, S], BF16, tag=f"qhT{p}", name=f"qhT{p}")
                   for p in range(NP)]
            khT = [act_pool.tile([128, S], BF16, tag=f"khT{p}",
                                 name=f"khT{p}") for p in range(NP)]
            vhs = [act_pool.tile([128, VW], BF16, tag=f"vh{t}", name=f"vh{t}")
                   for t in range(NKC)]

            # ---- issue order: q(qb0) path first ----
            for c in range(NC_CHUNKS):
                nc.sync.dma_start(wq_sb[:, c * 512:(c + 1) * 512],
                                  wq[c * 128:(c + 1) * 128, :])
            nc.sync.dma_start(mask_sb[:], maskr[:])
            nc.vector.memset(ones_sb[:], 1.0)
            nc.vector.memset(e65_sb[64:65, :], 1.0)
            for t in range(NKC):
                nc.vector.memset(vhs[t][:, :], 0.0)

            def stage_block(src, qb):
                stg = []
                for c in range(NC_CHUNKS):
                    t = st_pool.tile([128, 512], BF16, tag="stage",
                                     name=f"stg{c}")
                    nc.sync.dma_start(
                        t[:], src[c * 128:(c + 1) * 128,
                                  qb * 512:(qb + 1) * 512])
                    stg.append(t)
                return stg

            def proj_pair(stg, wsb, p, dst, qb):
                ps = pj_pool.tile([128, 512], F32, tag="pj")
                for c in range(NC_CHUNKS):
                    nc.tensor.matmul(
                        ps[:],
                        lhsT=wsb[:, c * 512 + p * 128:c * 512 + (p + 1) * 128],
                        rhs=stg[c][:],
                        start=(c == 0), stop=(c == NC_CHUNKS - 1))
                nc.vector.tensor_copy(
                    dst[p][:, qb * 512:(qb + 1) * 512], ps[:])

            # q(qb=0) projection
            stg = stage_block(qT, 0)
            for p in range(NP):
                proj_pair(stg, wq_sb, p, qhT, 0)

            # k projection (all blocks)
            for c in range(NC_CHUNKS):
                nc.sync.dma_start(wk_sb[:, c * 512:(c + 1) * 512],
                                  wk[c * 128:(c + 1) * 128, :])
            for kb in range(NQB):
                stg = stage_block(kT, kb)
                for p in range(NP):
                    proj_pair(stg, wk_sb, p, khT, kb)

            # v projection (with mask fold + ones col)
            for c in range(NC_CHUNKS):
                nc.sync.dma_start(wv_sb[:, c * 512:(c + 1) * 512],
                                  wv[c * 128:(c + 1) * 128, :])
            for p in range(NP):
                nc.sync.dma_start(wo_sb[:, p * 1024:(p + 1) * 1024],
                                  wo[p * 128:(p + 1) * 128, :])
            vt_sb = []
            for c in range(NC_CHUNKS):
                t = vt_pool.tile([128, S], BF16, tag=f"vt{c}", name=f"vt{c}")
                for tb in range(NQB):
                    nc.sync.dma_start(
                        t[:, tb * 512:(tb + 1) * 512],
                        vT[c * 128:(c + 1) * 128, tb * 512:(tb + 1) * 512])
                vt_sb.append(t)
            for t in range(NKC):
                ps = pj_pool.tile([128, 512], F32, tag="pj")
                for c in range(NC_CHUNKS):
                    nc.tensor.matmul(
                        ps[:],
                        lhsT=vt_sb[c][:, t * 128:(t + 1) * 128],
                        rhs=wv_sb[:, c * 512:(c + 1) * 512],
                        start=(c == 0), stop=(c == NC_CHUNKS - 1))
                # masked copy into vh store (strided per head) + mask col
                dst_dv = vhs[t][:, 0:VW].rearrange(
                    "p (h x) -> p h x", x=128)[:, :, 0:DV]
                src_dv = ps[:].rearrange("p (h x) -> p h x", x=DV)
                nc.vector.tensor_scalar_mul(dst_dv, src_dv,
                                            mask_sb[:, t:t + 1])
                dst_m = vhs[t][:, 0:VW].rearrange(
                    "p (h x) -> p h x", x=128)[:, :, DV:DV + 1]
                src_m = ones_sb[:, 0:HC].rearrange("p (h x) -> p h x", x=1)
                nc.vector.tensor_scalar_mul(dst_m, src_m,
                                            mask_sb[:, t:t + 1])

            # ---- attention + output projection ----
            # Software-pipelined over flat units u = (qb, p, kc): the
            # scores+exp issue runs LOOK units ahead of the mix issue so
            # ScalarE keeps exp-ing across pair boundaries while the PE
            # absorbs normalize/proj/Wo work in its slack.
            LOOK = 2
            units = [(qb, p, kc) for qb in range(NQB) for p in range(NP)
                     for kc in range(NKC)]
            pend = {}
            mix_tiles = {}
            stg_nxt = {}
            normT = {qb: [] for qb in range(NQB)}

            def issue_sc(u):
                qb, p, kc = u
                qsl = slice(qb * 512, (qb + 1) * 512)
                ksl = slice(kc * 128, (kc + 1) * 128)
                scP = sc_pool.tile([128, 1024], F32, tag="sc")
                # 64x128 PE row tiling: both heads co-stream
                nc.tensor.matmul(
                    scP[:, 0:512],
                    lhsT=khT[p][0:64, ksl], rhs=qhT[p][0:64, qsl],
                    start=True, stop=True, tile_position=(0, 0))
                nc.tensor.matmul(
                    scP[:, 512:1024],
                    lhsT=khT[p][64:128, ksl], rhs=qhT[p][64:128, qsl],
                    start=True, stop=True, tile_position=(64, 0))
                exP = exp_pool.tile([128, 1024], BF16, tag="exp")
                nc.scalar.activation(exP[:], scP[:], EXP)
                pend[u] = exP

            def issue_mix(u):
                qb, p, kc = u
                h0, h1 = 2 * p, 2 * p + 1
                if kc == 0:
                    mix_tiles[(qb, p)] = mx_pool.tile(
                        [128, 1024], F32, tag="mx", name="mixPR")
                mixPR = mix_tiles[(qb, p)]
                exP = pend.pop(u)
                va = vhs[kc]
                st = (kc == 0)
                sp = (kc == NKC - 1)
                nc.tensor.matmul(
                    mixPR[:, 0:512], lhsT=va[:, h0 * 128:(h0 + 1) * 128],
                    rhs=exP[:, 0:512], start=st, stop=sp)
                nc.tensor.matmul(
                    mixPR[:, 512:1024], lhsT=va[:, h1 * 128:(h1 + 1) * 128],
                    rhs=exP[:, 512:1024], start=st, stop=sp)

            def normalize(qb, p):
                # evac mix PSUM to SBUF first (frees the banks fast), then
                # Z row -> bf16, K=1 PE bcast, reciprocal at base partition
                # 0 (custom-DVE ops misbehave at base partition 64), scale
                mixPR = mix_tiles.pop((qb, p))
                mloc = tmp_pool.tile([128, 1024], F32, tag="mloc")
                nc.vector.tensor_copy(mloc[:], mixPR[:])
                nt = norm_pool.tile([128, 512], BF16, tag="norm")
                normT[qb].append(nt)
                zrow = tmp_pool.tile([128, 1024], BF16, tag="zrow")
                nc.vector.tensor_copy(zrow[64:65, :], mloc[64:65, :])
                bc0 = pj_pool.tile([128, 512], F32, tag="pj")
                bc1 = pj_pool.tile([128, 512], F32, tag="pj")
                nc.tensor.matmul(
                    bc0[0:64, :], lhsT=e65_sb[64:65, 0:64],
                    rhs=zrow[64:65, 0:512], start=True, stop=True,
                    tile_position=(64, 0))
                nc.tensor.matmul(
                    bc1[0:64, :], lhsT=e65_sb[64:65, 0:64],
                    rhs=zrow[64:65, 512:1024], start=True, stop=True,
                    tile_position=(64, 0))
                rec0 = tmp_pool.tile([64, 512], F32, tag="rec")
                rec1 = tmp_pool.tile([64, 512], F32, tag="rec")
                nc.vector.reciprocal_approx_fast(rec0[:], bc0[0:64, :])
                nc.vector.reciprocal_approx_fast(rec1[:], bc1[0:64, :])
                nc.vector.tensor_mul(nt[0:64, :], mloc[0:64, 0:512], rec0[:])
                sh1 = tmp_pool.tile([64, 512], BF16, tag="sh1")
                nc.vector.tensor_mul(sh1[:], mloc[0:64, 512:1024], rec1[:])
                nc.sync.dma_start(nt[64:128, :], sh1[:])

            deferred = []

            def wo_piece(qb, tt, dh):
                def run():
                    wps = pj_pool.tile([128, 512], F32, tag="pj",
                                       name="wps")
                    for p in range(NP):
                        nc.tensor.matmul(
                            wps[:],
                            lhsT=normT[qb][p][:, tt * 128:(tt + 1) * 128],
                            rhs=wo_sb[:, p * 1024 + dh * 512:
                                      p * 1024 + (dh + 1) * 512],
                            start=(p == 0), stop=(p == NP - 1))
                    osb = out_pool.tile([128, 512], F32, tag="osb",
                                        name="osb")
                    nc.vector.tensor_copy(osb[:], wps[:])
                    nc.sync.dma_start(
                        out[qb * 512 + tt * 128:qb * 512 + (tt + 1) * 128,
                            dh * 512:(dh + 1) * 512], osb[:])
                return run

            def proj_piece(qb, p, cs):
                def run():
                    stg = stg_nxt[qb]
                    key = ("pps", qb, p)
                    if cs == 0:
                        mix_tiles[key] = pj_pool.tile(
                            [128, 512], F32, tag="pj", name="pps")
                    ps = mix_tiles[key]
                    for c in (cs, cs + 1):
                        nc.tensor.matmul(
                            ps[:],
                            lhsT=wq_sb[:, c * 512 + p * 128:
                                       c * 512 + (p + 1) * 128],
                            rhs=stg[c][:],
                            start=(c == 0), stop=(c == NC_CHUNKS - 1))
                    if cs + 2 == NC_CHUNKS:
                        nc.vector.tensor_copy(
                            qhT[p][:, qb * 512:(qb + 1) * 512], ps[:])
                        del mix_tiles[key]
                return run

            for i, u in enumerate(units):
                if i == 0:
                    for j in range(LOOK):
                        issue_sc(units[j])
                issue_mix(u)
                if i + LOOK < len(units):
                    issue_sc(units[i + LOOK])
                if deferred:
                    deferred.pop(0)()
                qb, p, kc = u
                if p == 0 and kc == 0 and qb + 1 < NQB:
                    stg_nxt[qb + 1] = stage_block(qT, qb + 1)
                if kc == NKC - 1:
                    normalize(qb, p)
                    if qb + 1 < NQB:
                        deferred.extend(
                            proj_piece(qb + 1, p, cs)
                            for cs in range(0, NC_CHUNKS, 2))
                    if p == NP - 1:
                        deferred.extend(wo_piece(qb, tt, dh)
                                        for tt in range(4) for dh in range(2))
            while deferred:
                deferred.pop(0)()

    nc.compile()
    return nc


def _get_nc():
    if "nc" not in _COMPILED:
        _COMPILED["nc"] = _build_nc()
    return _COMPILED["nc"]


def _shard_inputs(q, k, v, mask, Wq, Wk, Wv, Wo):
    """Build the per-core input maps (host-side layout prep)."""
    import ml_dtypes

    bf16 = ml_dtypes.bfloat16
    in_maps = []
    maskf = np.asarray(mask).astype(np.float32)
    q = np.asarray(q, np.float32)
    k = np.asarray(k, np.float32)
    v = np.asarray(v, np.float32)
    Wq = np.asarray(Wq, np.float32)
    Wk = np.asarray(Wk, np.float32)
    Wv = np.asarray(Wv, np.float32)
    Wo = np.asarray(Wo, np.float32)
    scale = np.float32(1.0 / np.sqrt(DK))
    for c in range(NCORES):
        b, hg = c // 2, c % 2
        hs = hg * HC
        m = {
            "qT": np.ascontiguousarray(q[b].T).astype(bf16),
            "kT": np.ascontiguousarray(k[b].T).astype(bf16),
            "vT": np.ascontiguousarray(v[b].T).astype(bf16),
            # head-major col blocks; fold 1/sqrt(dk) into Wq
            "wq": np.ascontiguousarray(
                Wq[hs:hs + HC].transpose(1, 0, 2).reshape(D, HC * DK) * scale
            ).astype(bf16),
            "wk": np.ascontiguousarray(
                Wk[hs:hs + HC].transpose(1, 0, 2).reshape(D, HC * DK)
            ).astype(bf16),
            "wv": np.ascontiguousarray(
                Wv[hs:hs + HC].transpose(1, 0, 2).reshape(D, HC * DV)
            ).astype(bf16),
            "wo": np.ascontiguousarray(Wo[hs * DV:(hs + HC) * DV]).astype(bf16),
            "maskr": np.ascontiguousarray(
                maskf[b].reshape(NKC, 128).T).astype(np.float32),
        }
        in_maps.append(m)
    return in_maps


def kernel(q, k, v, mask, Wq, Wk, Wv, Wo, _trace=False):
    from concourse.bass_utils import run_bass_kernel_spmd

    nc = _get_nc()
    in_maps = _shard_inputs(q, k, v, mask, Wq, Wk, Wv, Wo)
    res = run_bass_kernel_spmd(nc, in_maps, list(range(NCORES)),
                               trace=_trace)
    out = np.zeros((B, S, D), np.float32)
    for c in range(NCORES):
        out[c // 2] += res.results[c]["out"]
    if _trace:
        _COMPILED["last_result"] = res
    return out
